# revision 6
# baseline (speedup 1.0000x reference)
"""AttentionMixer kernel for 8 Trainium2 NeuronCores.

Sharding: data-parallel over (batch B=4) x (query-half NQ/2) -> 8 cores.
Each core computes, for its (b, half):
    q = meshT slice proj, k/v = pc proj (k/v work duplicated across the
    2 cores of a batch), masked softmax attention, Wo projection.
Layout is "transposed" throughout (features on partitions, tokens on the
free dim) so every matmul contracts over the partition dim natively:
    qT/kT: [e, n] via W.T as lhsT, xT as rhs
    scoresT: [nk, nq] = kT_h.T-contract-d qT_h  (2 heads row-packed)
    attnT = exp(scoresT/8 + mask_bias)          (one ACT op per tile)
    ctxT_h: [65, nq] via v_aug lhsT (ones column -> softmax denom Z for
    free), normalized post-hoc: mix = (attn@v)@Wo.T / Z + (Wo@bv + bo).
All big matmuls bf16 with fp32 PSUM accumulation.
"""

import numpy as np
import ml_dtypes

import concourse.bass as bass
import concourse.bacc as bacc
import concourse.mybir as mybir
import concourse.tile as tile
from concourse.bass_utils import run_bass_kernel_spmd

B, NQ, NK, E, DPC, H = 4, 2048, 4096, 256, 128, 4
HD = E // H  # 64
NQH = NQ // 2  # per-core queries: 1024
NKB = NK // 128  # 32 nk blocks
P = 128
BF16 = mybir.dt.bfloat16
F32 = mybir.dt.float32
MASK_NEG = -80.0

_CACHE = {}


def build_nc():
    nc = bacc.Bacc(None)

    # ---- DRAM params (per-core shapes; host stages exact SBUF layouts) ----
    meshT_d = nc.declare_dram_parameter("meshT", [2, P, NQH], BF16, False)       # [c_blk, c, nq]
    pcT_d = nc.declare_dram_parameter("pcT", [P, NK], BF16, False)               # [c, nk]
    wqT_d = nc.declare_dram_parameter("wqT", [2, P, E], BF16, False)             # [c_blk, c, e]
    wkT_d = nc.declare_dram_parameter("wkT", [P, E], BF16, False)                # [c, e]
    wvT_d = nc.declare_dram_parameter("wvT", [P, E], BF16, False)                # [c, e]
    woT_d = nc.declare_dram_parameter("woT", [H, HD, E], BF16, False)            # [h, e_loc, e']
    bq_d = nc.declare_dram_parameter("bq2", [P, 2], F32, False)                  # [e_loc, e_blk]
    bk_d = nc.declare_dram_parameter("bk2", [P, 2], F32, False)
    bop_d = nc.declare_dram_parameter("bop2", [P, 2], F32, False)                # Wo@bv + bo
    maskb_d = nc.declare_dram_parameter("maskb", [P, NKB], F32, False)           # exp bias
    mixT_d = nc.declare_dram_parameter("mixT", [2, P, NQH], F32, isOutput=True)

    with tile.TileContext(nc) as tc:
        with (
            tc.tile_pool(name="const", bufs=1) as cpool,
            tc.tile_pool(name="acts", bufs=1) as apool,
            tc.tile_pool(name="attn", bufs=3) as attn_pool,
            tc.tile_pool(name="small", bufs=2) as spool,
            tc.tile_pool(name="ps_big", bufs=2, space="PSUM") as ps_big,
            tc.tile_pool(name="ps_ctx", bufs=3, space="PSUM") as ps_ctx,
            tc.tile_pool(name="ps_zb", bufs=1, space="PSUM") as ps_zb,
        ):
            # ---- load constants / inputs into SBUF ----
            meshT = cpool.tile([P, 2, NQH], BF16)
            pcT = cpool.tile([P, NK], BF16)
            wqT = cpool.tile([P, 2, E], BF16)
            wkT = cpool.tile([P, E], BF16)
            wvT = cpool.tile([P, E], BF16)
            woT = cpool.tile([HD, H, E], BF16)
            bq = cpool.tile([P, 2], F32)
            bk = cpool.tile([P, 2], F32)
            bop = cpool.tile([P, 2], F32)
            maskb = cpool.tile([P, NKB], F32)
            ones_f = cpool.tile([1, HD], F32)

            for cb in range(2):
                nc.sync.dma_start(meshT[:, cb, :], meshT_d[cb])
                nc.sync.dma_start(wqT[:, cb, :], wqT_d[cb])
            nc.sync.dma_start(pcT[:], pcT_d[:, :])
            nc.sync.dma_start(wkT[:], wkT_d[:, :])
            nc.sync.dma_start(wvT[:], wvT_d[:, :])
            for h in range(H):
                nc.sync.dma_start(woT[:, h, :], woT_d[h])
            nc.sync.dma_start(bq[:], bq_d[:, :])
            nc.sync.dma_start(bk[:], bk_d[:, :])
            nc.sync.dma_start(bop[:], bop_d[:, :])
            nc.sync.dma_start(maskb[:], maskb_d[:, :])
            nc.vector.memset(ones_f[:], 1.0)

            # ---- projections ----
            # qT[e, nq]: lhsT = wqT[cb][:, e_blk], rhs = meshT[cb]
            qT = apool.tile([P, 2, NQH], BF16)
            for eb in range(2):
                for nt in range(NQH // 512):
                    ps = ps_big.tile([P, 1024], F32, tag="big")
                    for cb in range(2):
                        nc.tensor.matmul(
                            ps[:, 0:512],
                            wqT[:, cb, eb * P:(eb + 1) * P],
                            meshT[:, cb, nt * 512:(nt + 1) * 512],
                            start=(cb == 0), stop=(cb == 1),
                        )
                    nc.scalar.add(
                        qT[:, eb, nt * 512:(nt + 1) * 512], ps[:, 0:512],
                        bq[:, eb:eb + 1])

            # kT[e, nk]: lhsT = wkT[:, e_blk], rhs = pcT
            kT = apool.tile([P, 2, NK], BF16)
            for eb in range(2):
                for nt in range(NK // 512):
                    ps = ps_big.tile([P, 1024], F32, tag="big")
                    nc.tensor.matmul(
                        ps[:, 0:512],
                        wkT[:, eb * P:(eb + 1) * P],
                        pcT[:, nt * 512:(nt + 1) * 512],
                        start=True, stop=True,
                    )
                    nc.scalar.add(
                        kT[:, eb, nt * 512:(nt + 1) * 512], ps[:, 0:512],
                        bk[:, eb:eb + 1])

            # v (natural layout, no bias: folded into bop) with ones column
            # per head slot: v_sb[:, j, h*65 : h*65+64] = v, [.. +64] = 1.0
            v_sb = apool.tile([P, NKB, H * (HD + 1)], BF16)
            for h in range(H):
                nc.vector.memset(v_sb[:, :, h * 65 + 64:h * 65 + 65], 1.0)
            for j in range(NKB):
                ps = ps_big.tile([P, 1024], F32, tag="big")
                nc.tensor.matmul(
                    ps[:, 0:E],
                    pcT[:, j * P:(j + 1) * P],
                    wvT[:],
                    start=True, stop=True,
                )
                vdst = v_sb[:, j, :].rearrange("p (h x) -> p h x", x=HD + 1)
                nc.vector.tensor_copy(
                    vdst[:, :, 0:HD],
                    ps[:, 0:E].rearrange("p (h x) -> p h x", x=HD))

            # ---- attention main loop ----
            ctxn = apool.tile([HD, H, NQH], BF16)  # normalized ctxT per head
            for hp in range(2):          # head pair: heads 2hp, 2hp+1
                for nt in range(NQH // 512):   # nq half
                    h0, h1 = 2 * hp, 2 * hp + 1
                    acc0 = ps_ctx.tile([HD + 1, 512], F32, tag="ctx")
                    acc1 = ps_ctx.tile([HD + 1, 512], F32, tag="ctx")
                    for j in range(NKB):
                        s = ps_big.tile([P, 1024], F32, tag="big")
                        # scores for the two heads -> adjacent psum banks
                        nc.tensor.matmul(
                            s[:, 0:512],
                            kT[0:HD, hp, j * P:(j + 1) * P],
                            qT[0:HD, hp, nt * 512:(nt + 1) * 512],
                            start=True, stop=True,
                        )
                        nc.tensor.matmul(
                            s[:, 512:1024],
                            kT[HD:P, hp, j * P:(j + 1) * P],
                            qT[HD:P, hp, nt * 512:(nt + 1) * 512],
                            start=True, stop=True,
                        )
                        a = attn_pool.tile([P, 1024], BF16, tag="attn")
                        nc.scalar.activation(
                            a[:], s[:],
                            mybir.ActivationFunctionType.Exp,
                            bias=maskb[:, j:j + 1], scale=0.125)
                        nc.tensor.matmul(
                            acc0[:],
                            v_sb[:, j, h0 * 65:(h0 + 1) * 65],
                            a[:, 0:512],
                            start=(j == 0), stop=(j == NKB - 1),
                        )
                        nc.tensor.matmul(
                            acc1[:],
                            v_sb[:, j, h1 * 65:(h1 + 1) * 65],
                            a[:, 512:1024],
                            start=(j == 0), stop=(j == NKB - 1),
                        )
                    # normalize: ctx[0:64] / Z (Z = row 64)
                    for h, acc in ((h0, acc0), (h1, acc1)):
                        zr = spool.tile([1, 512], F32, tag="zr")
                        nc.vector.reciprocal(zr[:], acc[HD:HD + 1, :])
                        zb = ps_zb.tile([HD, 512], F32, tag="zb")
                        nc.tensor.matmul(zb[:], ones_f[:], zr[:],
                                         start=True, stop=True)
                        # DVE has a single PSUM read port: stage zb in SBUF
                        zbs = spool.tile([HD, 512], F32, tag="zbs")
                        nc.vector.tensor_copy(zbs[:], zb[:])
                        nc.vector.tensor_mul(
                            ctxn[:, h, nt * 512:(nt + 1) * 512],
                            acc[0:HD, :], zbs[:])

            # ---- output projection: mixT[e'] = sum_h WoT_h.T @ ctxn_h ----
            mixT = apool.tile([P, 2, NQH], F32)
            for eb in range(2):
                for nt in range(NQH // 512):
                    ps = ps_big.tile([P, 1024], F32, tag="big")
                    for h in range(H):
                        nc.tensor.matmul(
                            ps[:, 0:512],
                            woT[:, h, eb * P:(eb + 1) * P],
                            ctxn[:, h, nt * 512:(nt + 1) * 512],
                            start=(h == 0), stop=(h == H - 1),
                        )
                    nc.scalar.add(
                        mixT[:, eb, nt * 512:(nt + 1) * 512], ps[:, 0:512],
                        bop[:, eb:eb + 1])
            for eb in range(2):
                nc.sync.dma_start(mixT_d[eb], mixT[:, eb, :])

    nc.finalize()
    return nc


def _get_nc():
    if "nc" not in _CACHE:
        _CACHE["nc"] = build_nc()
    return _CACHE["nc"]


def kernel(mesh_feats, pc_feats, Wq, Wk, Wv, bq, bk, bv, Wo, bo, lengths,
           _trace=False, _trace_kwargs=None):
    mesh_feats = np.asarray(mesh_feats, np.float32)
    pc_feats = np.asarray(pc_feats, np.float32)
    Wq, Wk, Wv = (np.asarray(x, np.float32) for x in (Wq, Wk, Wv))
    bqv, bkv, bvv = (np.asarray(x, np.float32) for x in (bq, bk, bv))
    Wo, bo = np.asarray(Wo, np.float32), np.asarray(bo, np.float32)
    lengths = np.asarray(lengths, np.int32)

    bf = ml_dtypes.bfloat16
    wqT = np.ascontiguousarray(Wq.T.reshape(2, P, E)).astype(bf)
    wkT = np.ascontiguousarray(Wk.T).astype(bf)          # [128, 256]
    wvT = np.ascontiguousarray(Wv.T).astype(bf)          # [128, 256]
    woT = np.ascontiguousarray(Wo.T.reshape(H, HD, E)).astype(bf)
    bq2 = np.ascontiguousarray(bqv.reshape(2, P).T)      # [128, 2]
    bk2 = np.ascontiguousarray(bkv.reshape(2, P).T)
    bop = Wo @ bvv + bo
    bop2 = np.ascontiguousarray(bop.reshape(2, P).T)

    idx = np.arange(NK).reshape(NKB, P).T                # [128, 32]
    in_maps = []
    for c in range(8):
        b, half = c // 2, c % 2
        meshT = np.ascontiguousarray(
            mesh_feats[b, half * NQH:(half + 1) * NQH, :].T
        ).reshape(2, P, NQH).astype(bf)
        pcT = np.ascontiguousarray(pc_feats[b].T).astype(bf)
        maskb = np.where(idx < int(lengths[b]), 0.0, MASK_NEG).astype(np.float32)
        maskb = np.ascontiguousarray(maskb)
        in_maps.append({
            "meshT": meshT, "pcT": pcT, "wqT": wqT, "wkT": wkT,
            "wvT": wvT, "woT": woT, "bq2": bq2, "bk2": bk2,
            "bop2": bop2, "maskb": maskb,
        })

    nc = _get_nc()
    res = run_bass_kernel_spmd(
        nc, in_maps, list(range(8)),
        trace=_trace, **(_trace_kwargs or {}))
    out = np.empty((B, NQ, 2 * E), np.float32)
    out[:, :, :E] = mesh_feats
    for c in range(8):
        b, half = c // 2, c % 2
        mixT = res.results[c]["mixT"]            # [2, 128, NQH]
        out[b, half * NQH:(half + 1) * NQH, E:] = mixT.reshape(E, NQH).T
    if _trace:
        return out, res
    return out
